# revision 19
# baseline (speedup 1.0000x reference)
"""Trainium2 Bass kernel for nn_ItemAutoencoder (LSTM autoencoder).

Model: x[B,T,D] -> relu(x @ in_W.T + in_b)            [B,T,64]
         -> LSTM(64->256) -> LSTM(256->256)            [B,T,256]
         -> z = h[:, -1]                               [B,256]
         -> repeat z over T -> LSTM(256->64) -> LSTM(64->64)
         -> out = d @ out_W.T + out_b                  [B,T,256]
B=1024, T=100, D=256.  Sharding: data-parallel, batch 128 per core x 8 cores.

Per-core structure (all matmuls bf16):
  - Recurrent state h kept TRANSPOSED in bf16 ([128, H] tiles, H rows packed
    into 2x128-col chunks) as the stationary operand of the gate matmuls:
    gates[B, 4H] = hT.T @ W.
  - Gate order host-permuted to [f, i, o, g]: sigmoid covers one contiguous
    col range, tanh the tail.
  - Encoder L0 bias rides a ones row in the in-proj output (K=65).
    Encoder L1 bias is pre-written into PSUM by the DVE (TensorCopy from a
    host-shipped [128, 4H] f32 broadcast tile); the gate matmuls then
    accumulate with start=False.
  - Elementwise ops use scalar_tensor_tensor (2x DVE rate, all-SBUF fp32).
  - h is produced in bf16, transposed on the PE in bf16 (1 cycle/row vs 4 for
    fp32), and copied PSUM->SBUF by one DVE TensorCopy (2x, 2-byte).
  - Decoder: L0 input (z@dW0i + b, const over t) re-injected per step via an
    identity matmul; dec L1/out biases ride ones rows in the dT state tiles.
    Output projection DMAs straight from PSUM to DRAM each step.
"""
import os
import numpy as np
from contextlib import ExitStack

import concourse.bass as bass
import concourse.tile as tile
from concourse import bacc, mybir
from concourse import bass_utils

F32 = mybir.dt.float32
BF16 = mybir.dt.bfloat16
import ml_dtypes
MM_DT = BF16
MM_NP = ml_dtypes.bfloat16
AF = mybir.ActivationFunctionType
ALU = mybir.AluOpType
TS = bass.ts

N_CORES = 8
B = 128            # per-core batch
T = 100
D = 256
H = 256            # encoder hidden
M = 64             # in-proj dim / decoder hidden
GE = 4 * H         # 1024
GD = 4 * M         # 256

_CACHE: dict = {}


def build_nc():
    nc = bacc.Bacc("TRN2", target_bir_lowering=False, debug=False)

    # ---- DRAM I/O -----------------------------------------------------------
    xT_d = nc.dram_tensor("xT", [2, 128, T * B], MM_DT, kind="ExternalInput")
    inWT_d = nc.dram_tensor("inWT", [2, 128, M], MM_DT, kind="ExternalInput")
    inb_d = nc.dram_tensor("inb", [M, 1], F32, kind="ExternalInput")
    w0in_d = nc.dram_tensor("w0in", [M + 1, GE], MM_DT, kind="ExternalInput")
    w0rec_d = nc.dram_tensor("w0rec", [2, 128, GE], MM_DT, kind="ExternalInput")
    w1in_d = nc.dram_tensor("w1in", [2, 128, GE], MM_DT, kind="ExternalInput")
    w1rec_d = nc.dram_tensor("w1rec", [2, 128, GE], MM_DT, kind="ExternalInput")
    b1bc_d = nc.dram_tensor("b1bc", [128, GE], F32, kind="ExternalInput")
    dw0in_d = nc.dram_tensor("dw0in", [2, 128, GD], MM_DT, kind="ExternalInput")
    bd0_d = nc.dram_tensor("bd0", [1, GD], MM_DT, kind="ExternalInput")
    dw0rec_d = nc.dram_tensor("dw0rec", [M, GD], MM_DT, kind="ExternalInput")
    dw1in_d = nc.dram_tensor("dw1in", [M + 1, GD], MM_DT, kind="ExternalInput")
    dw1rec_d = nc.dram_tensor("dw1rec", [M, GD], MM_DT, kind="ExternalInput")
    wout_d = nc.dram_tensor("wout", [M + 1, D], MM_DT, kind="ExternalInput")
    identr_d = nc.dram_tensor("identr", [128, 128], MM_DT, kind="ExternalInput")
    ones1_d = nc.dram_tensor("ones1", [1, 128], MM_DT, kind="ExternalInput")
    out_d = nc.dram_tensor("out", [B, T * D], F32, kind="ExternalOutput")

    with tile.TileContext(nc) as tc, ExitStack() as ctx:
        P = bass.MemorySpace.PSUM
        wp = ctx.enter_context(tc.tile_pool(name="w", bufs=1))

        def wtile(dram_ap, shape, tag, dt=MM_DT):
            t_ = wp.tile(shape, dt, tag=tag)
            nc.sync.dma_start(t_[:], dram_ap)
            return t_

        # ---- persistent weights in SBUF ------------------------------------
        inWT = [wtile(inWT_d[kb, :, :], [128, M], f"inWT{kb}") for kb in range(2)]
        inb = wtile(inb_d[:], [M, 1], "inb", F32)
        w0in = wtile(w0in_d[:], [M + 1, GE], "w0in")
        w0rec = [wtile(w0rec_d[kb, :, :], [128, GE], f"w0rec{kb}") for kb in range(2)]
        w1in = [wtile(w1in_d[kb, :, :], [128, GE], f"w1in{kb}") for kb in range(2)]
        w1rec = [wtile(w1rec_d[kb, :, :], [128, GE], f"w1rec{kb}") for kb in range(2)]
        b1bc = wtile(b1bc_d[:], [128, GE], "b1bc", F32)
        dw0in = [wtile(dw0in_d[kb, :, :], [128, GD], f"dw0in{kb}") for kb in range(2)]
        bd0 = wtile(bd0_d[:], [1, GD], "bd0")
        dw0rec = wtile(dw0rec_d[:], [M, GD], "dw0rec")
        dw1in = wtile(dw1in_d[:], [M + 1, GD], "dw1in")
        dw1rec = wtile(dw1rec_d[:], [M, GD], "dw1rec")
        wout = wtile(wout_d[:], [M + 1, D], "wout")
        identr = wtile(identr_d[:], [128, 128], "identr")
        ones1 = wtile(ones1_d[:], [1, 128], "ones1")

        # in-proj output, transposed, with a ones row (row 64) for bias riding
        h0aug_h = nc.alloc_sbuf_tensor("h0aug", [M + 1, T * B], MM_DT)
        h0aug = h0aug_h.ap()
        nc.gpsimd.memset(h0aug[M : M + 1, :], 1.0)

        # ============================= in-proj ==============================
        xpool = ctx.enter_context(tc.tile_pool(name="xc", bufs=4))
        with tc.tile_pool(name="psip", bufs=2, space=P) as psip:
            NG = T * B // 512  # 25
            for g in range(NG):
                xa = xpool.tile([128, 512], MM_DT, tag="xa")
                xb = xpool.tile([128, 512], MM_DT, tag="xb")
                nc.sync.dma_start(xa[:], xT_d[0, :, TS(g, 512)])
                nc.sync.dma_start(xb[:], xT_d[1, :, TS(g, 512)])
                ps = psip.tile([M, 512], F32)
                nc.tensor.matmul(ps[:], inWT[0][:], xa[:], start=True, stop=False)
                nc.tensor.matmul(ps[:], inWT[1][:], xb[:], start=False, stop=True)
                nc.scalar.activation(
                    h0aug[0:M, TS(g, 512)], ps[:], AF.Relu, bias=inb[:, 0:1]
                )

        # ============================= encoder ==============================
        gpool = ctx.enter_context(tc.tile_pool(name="g", bufs=2))
        spool = ctx.enter_context(tc.tile_pool(name="s", bufs=2))

        # persistent state tiles: [A, B, Z] rotation, Z = zero-init (t=-1)
        def zstate(tag, shape, dt):
            t_ = nc.alloc_sbuf_tensor(tag, shape, dt).ap()
            nc.gpsimd.memset(t_[:], 0.0)
            return t_

        hT = {
            l: [zstate(f"h{l}T{i}", [128, H], MM_DT) for i in range(3)]
            for l in range(2)
        }
        cst = {l: [zstate(f"c{l}_{i}", [128, H], BF16) for i in range(3)] for l in range(2)}

        with (
            tc.tile_pool(name="pg0", bufs=1, space=P) as pg0p,
            tc.tile_pool(name="pg1", bufs=2, space=P) as pg1p,
            tc.tile_pool(name="ptr", bufs=2, space=P) as ptrp,
        ):
            pg = {0: pg0p.tile([128, GE], F32, tag="pg0", name="pg0"), 1: None}

            st = {0: {}, 1: {}}

            def enc_prefill_b1():
                # write L1 bias into PSUM; gate matmuls accumulate on top.
                # bufs=2 rotation: this only waits on sigma reads from 2 slots ago,
                # giving the (slow, idle) gpsimd engine a full slot of slack.
                pg[1] = pg1p.tile([128, GE], F32, tag="pg1", name="pg1")
                nc.vector.tensor_copy(pg[1][:], b1bc[:])

            def enc_mms(l, t):
                ps = pg[l]
                if l == 0:
                    srcs = [(h0aug[:, TS(t, 128)], w0in)]
                    hprev = hT[0][(t - 1) % 3 if t >= 1 else 2]
                    wrec = w0rec
                else:
                    hin = hT[0][t % 3]
                    srcs = [(hin[:, 0:128], w1in[0]), (hin[:, 128:256], w1in[1])]
                    hprev = hT[1][(t - 1) % 3 if t >= 1 else 2]
                    wrec = w1rec
                srcs += [(hprev[:, 0:128], wrec[0]), (hprev[:, 128:256], wrec[1])]
                first = l == 0  # L1 accumulates onto DVE-prefilled bias
                for chunk in range(2):
                    ns = slice(chunk * 512, (chunk + 1) * 512)
                    for j, (lh, rh) in enumerate(srcs):
                        nc.tensor.matmul(
                            ps[:, ns],
                            lh,
                            rh[:, ns],
                            start=(j == 0 and first),
                            stop=(j == len(srcs) - 1),
                            skip_group_check=not first,
                        )

            def enc_act_gates(l):
                ps = pg[l]
                gsb = gpool.tile([128, GE], BF16, tag=f"gsb{l}")
                st[l]["gsb"] = gsb
                # layout [f, i, o, g]: split sigmoid so f lands early
                nc.scalar.activation(gsb[:, 0 : 2 * H], ps[:, 0 : 2 * H], AF.Sigmoid)
                nc.scalar.activation(gsb[:, 3 * H :], ps[:, 3 * H :], AF.Tanh)
                nc.scalar.activation(gsb[:, 2 * H : 3 * H], ps[:, 2 * H : 3 * H], AF.Sigmoid)

            def enc_dve_c(l, t):
                s = st[l]
                gsb = s["gsb"]
                c_prev = cst[l][(t - 1) % 3 if t >= 1 else 2]
                c_new = cst[l][t % 3]
                ctmp = spool.tile([128, H], BF16, tag=f"ctmp{l}")
                u = spool.tile([128, H], BF16, tag=f"u{l}")
                nc.vector.scalar_tensor_tensor(
                    ctmp[:], gsb[:, 0:H], 1.0, c_prev[:], ALU.mult, ALU.mult
                )
                nc.vector.scalar_tensor_tensor(
                    u[:], gsb[:, H : 2 * H], 1.0, gsb[:, 3 * H :], ALU.mult, ALU.mult
                )
                nc.vector.scalar_tensor_tensor(
                    c_new[:], ctmp[:], 1.0, u[:], ALU.mult, ALU.add
                )
                s["c"] = c_new

            def enc_act_tanhc(l):
                s = st[l]
                tcs = spool.tile([128, H], BF16, tag=f"tc{l}")
                nc.scalar.activation(tcs[:], s["c"][:], AF.Tanh)
                s["tc"] = tcs

            def enc_dve_h(l):
                s = st[l]
                hsb = spool.tile([128, H], MM_DT, tag=f"hsb{l}")
                nc.vector.scalar_tensor_tensor(
                    hsb[:], s["gsb"][:, 2 * H : 3 * H], 1.0, s["tc"][:], ALU.mult, ALU.mult
                )
                s["h"] = hsb

            def enc_pe_tr(l):
                s = st[l]
                ptr = ptrp.tile([128, H], MM_DT, tag="ptr")
                nc.tensor.transpose(ptr[:, 0:128], s["h"][:, 0:128], identr[:])
                nc.tensor.transpose(ptr[:, 128:256], s["h"][:, 128:256], identr[:])
                s["ptr"] = ptr

            def enc_copy_h(l, t):
                s = st[l]
                nc.vector.tensor_copy(hT[l][t % 3][:], s["ptr"][:])

            def enc_slot(work):  # work = list of (layer, t); L0 first
                if any(l == 1 for l, _ in work):
                    enc_prefill_b1()
                for l, t in work:
                    enc_mms(l, t)
                for l, _ in work:
                    enc_act_gates(l)
                for l, t in work:
                    enc_dve_c(l, t)
                for l, _ in work:
                    enc_act_tanhc(l)
                for l, _ in work:
                    enc_dve_h(l)
                for l, _ in work:
                    enc_pe_tr(l)
                for l, t in work:
                    enc_copy_h(l, t)

            for t in range(T):
                work = [(0, t)]
                if t >= 1:
                    work.append((1, t - 1))
                enc_slot(work)
            enc_slot([(1, T - 1)])

        zT = hT[1][(T - 1) % 3]  # [128, 256] bf16

        # ============================= decoder ==============================
        dst = {}
        with (
            tc.tile_pool(name="pd0", bufs=2, space=P) as pd0p,
            tc.tile_pool(name="pd1", bufs=1, space=P) as pd1p,
            tc.tile_pool(name="ptrd", bufs=2, space=P) as ptrdp,
            tc.tile_pool(name="pout", bufs=2, space=P) as poutp,
            tc.tile_pool(name="pxg", bufs=1, space=P) as pxgp,
        ):
            # xg0 = z @ dW0i.T + bd0  (constant over time); kept f32 in SBUF and
            # re-injected into PSUM each step by the gpsimd engine
            psx = pxgp.tile([128, GD], F32)
            nc.tensor.matmul(psx[:], ones1[:], bd0[:], start=True, stop=False)
            nc.tensor.matmul(psx[:], zT[:, 0:128], dw0in[0][:], start=False, stop=False)
            nc.tensor.matmul(psx[:], zT[:, 128:256], dw0in[1][:], start=False, stop=True)
            xg0 = wp.tile([128, GD], F32, tag="xg0")
            nc.scalar.activation(xg0[:], psx[:], AF.Copy)

            # persistent ones-row-augmented transposed decoder states [A,B,Z]
            def dstate(tag):
                t_ = nc.alloc_sbuf_tensor(tag, [M + 1, 128], MM_DT).ap()
                nc.gpsimd.memset(t_[:], 0.0)
                nc.gpsimd.memset(t_[M : M + 1, :], 1.0)
                return t_

            dT = {l: [dstate(f"d{l}T{i}") for i in range(3)] for l in range(2)}
            dcs = {l: [zstate(f"cd{l}_{i}", [128, M], BF16) for i in range(3)] for l in range(2)}

            pd = {0: None, 1: pd1p.tile([128, GD], F32, tag="pd1", name="pd1")}

            def dec_prefill_xg0():
                pd[0] = pd0p.tile([128, GD], F32, tag="pd0", name="pd0")
                nc.vector.tensor_copy(pd[0][:], xg0[:])

            def dec_mms(l, t):
                ps = pd[l]
                dprev = dT[l][(t - 1) % 3 if t >= 1 else 2]
                if l == 0:
                    nc.tensor.matmul(
                        ps[:], dprev[0:M, :], dw0rec[:],
                        start=False, stop=True, skip_group_check=True,
                    )
                else:
                    din = dT[0][t % 3]
                    nc.tensor.matmul(ps[:], din[0 : M + 1, :], dw1in[:], start=True, stop=False)
                    nc.tensor.matmul(ps[:], dprev[0:M, :], dw1rec[:], start=False, stop=True)

            def dec_act_gates(l):
                ps = pd[l]
                gsb = gpool.tile([128, GD], BF16, tag=f"dgsb{l}")
                dst[l]["gsb"] = gsb
                nc.scalar.activation(gsb[:, 0 : 3 * M], ps[:, 0 : 3 * M], AF.Sigmoid)
                nc.scalar.activation(gsb[:, 3 * M :], ps[:, 3 * M :], AF.Tanh)

            def dec_dve_c(l, t):
                s = dst[l]
                gsb = s["gsb"]
                c_prev = dcs[l][(t - 1) % 3 if t >= 1 else 2]
                c_new = dcs[l][t % 3]
                ctmp = spool.tile([128, M], BF16, tag=f"dctmp{l}")
                u = spool.tile([128, M], BF16, tag=f"du{l}")
                nc.vector.scalar_tensor_tensor(
                    ctmp[:], gsb[:, 0:M], 1.0, c_prev[:], ALU.mult, ALU.mult
                )
                nc.vector.scalar_tensor_tensor(
                    u[:], gsb[:, M : 2 * M], 1.0, gsb[:, 3 * M :], ALU.mult, ALU.mult
                )
                nc.vector.scalar_tensor_tensor(
                    c_new[:], ctmp[:], 1.0, u[:], ALU.mult, ALU.add
                )
                s["c"] = c_new

            def dec_act_tanhc(l):
                s = dst[l]
                tcs = spool.tile([128, M], BF16, tag=f"dtc{l}")
                nc.scalar.activation(tcs[:], s["c"][:], AF.Tanh)
                s["tc"] = tcs

            def dec_dve_h(l):
                s = dst[l]
                hsb = spool.tile([128, M], MM_DT, tag=f"dhsb{l}")
                nc.vector.scalar_tensor_tensor(
                    hsb[:], s["gsb"][:, 2 * M : 3 * M], 1.0, s["tc"][:], ALU.mult, ALU.mult
                )
                s["h"] = hsb

            def dec_pe_tr(l):
                s = dst[l]
                ptr = ptrdp.tile([M, 128], MM_DT, tag="ptrd")
                nc.tensor.transpose(ptr[:], s["h"][:], identr[:])
                s["ptr"] = ptr

            def dec_copy_h(l, t):
                s = dst[l]
                nc.vector.tensor_copy(dT[l][t % 3][0:M, :], s["ptr"][:])

            ochunk = {"tile": None}

            def outproj_mm(t):
                d1T = dT[1][t % 3]
                ps = poutp.tile([128, D], F32, tag="pout")
                nc.tensor.matmul(ps[:], d1T[0 : M + 1, :], wout[:], start=True, stop=True)
                if t % 10 == 0:
                    ochunk["tile"] = spool.tile([128, 10 * D], F32, tag="ochunk", name="ochunk")
                dest = ochunk["tile"][:, TS(t % 10, D)]
                if t % 2 == 0:
                    nc.scalar.activation(dest, ps[:], AF.Copy)
                else:
                    nc.vector.tensor_copy(dest, ps[:])
                if t % 10 == 9:
                    nc.sync.dma_start(out_d[:, TS(t // 10, 10 * D)], ochunk["tile"][:])

            def dec_slot(work, op_t):
                if any(l == 0 for l, _ in work):
                    dec_prefill_xg0()
                for l, t in work:
                    dst.setdefault(l, {})
                    dec_mms(l, t)
                if op_t is not None:
                    outproj_mm(op_t)
                for l, _ in work:
                    dec_act_gates(l)
                for l, t in work:
                    dec_dve_c(l, t)
                for l, _ in work:
                    dec_act_tanhc(l)
                for l, _ in work:
                    dec_dve_h(l)
                for l, _ in work:
                    dec_pe_tr(l)
                for l, t in work:
                    dec_copy_h(l, t)

            for t in range(T):
                work = [(0, t)]
                if t >= 1:
                    work.append((1, t - 1))
                op_t = t - 2 if t >= 2 else None
                dec_slot(work, op_t)
            dec_slot([(1, T - 1)], T - 2)
            dec_slot([], T - 1)

    nc.compile()
    return nc


# ----------------------------------------------------------------------------
# host-side wrapper
# ----------------------------------------------------------------------------

def _perm(n):
    """pytorch gate order i,f,g,o (blocks of n) -> [f, i, o, g]."""
    idx = np.arange(4 * n).reshape(4, n)
    return np.concatenate([idx[1], idx[0], idx[3], idx[2]])


def _prep_core_inputs(inputs, core):
    f = np.float32
    pe = _perm(H)
    pd = _perm(M)
    x = inputs["x"][core * B : (core + 1) * B]          # [128, 100, 256]
    xT = np.ascontiguousarray(x.transpose(2, 1, 0)).reshape(2, 128, T * B)

    w0in = np.concatenate(
        [inputs["eW0i"].T[:, pe], (inputs["eb0i"] + inputs["eb0h"])[None, pe]], 0
    )
    w0rec = inputs["eW0h"].T[:, pe].reshape(2, 128, GE)
    w1in = inputs["eW1i"].T[:, pe].reshape(2, 128, GE)
    w1rec = inputs["eW1h"].T[:, pe].reshape(2, 128, GE)
    b1bc = np.broadcast_to((inputs["eb1i"] + inputs["eb1h"])[None, pe], (128, GE))
    dw0in = inputs["dW0i"].T[:, pd].reshape(2, 128, GD)
    bd0 = (inputs["db0i"] + inputs["db0h"])[None, pd]
    dw0rec = inputs["dW0h"].T[:, pd]
    dw1in = np.concatenate(
        [inputs["dW1i"].T[:, pd], (inputs["db1i"] + inputs["db1h"])[None, pd]], 0
    )
    dw1rec = inputs["dW1h"].T[:, pd]
    wout = np.concatenate([inputs["out_W"].T, inputs["out_b"][None, :]], 0)

    g = MM_NP
    return {
        "xT": np.ascontiguousarray(xT, dtype=g),
        "inWT": np.ascontiguousarray(inputs["in_W"].T.reshape(2, 128, M), dtype=g),
        "inb": np.ascontiguousarray(inputs["in_b"][:, None], dtype=f),
        "w0in": np.ascontiguousarray(w0in, dtype=g),
        "w0rec": np.ascontiguousarray(w0rec, dtype=g),
        "w1in": np.ascontiguousarray(w1in, dtype=g),
        "w1rec": np.ascontiguousarray(w1rec, dtype=g),
        "b1bc": np.ascontiguousarray(b1bc, dtype=f),
        "dw0in": np.ascontiguousarray(dw0in, dtype=g),
        "bd0": np.ascontiguousarray(bd0, dtype=g),
        "dw0rec": np.ascontiguousarray(dw0rec, dtype=g),
        "dw1in": np.ascontiguousarray(dw1in, dtype=g),
        "dw1rec": np.ascontiguousarray(dw1rec, dtype=g),
        "wout": np.ascontiguousarray(wout, dtype=g),
        "identr": np.eye(128).astype(g),
        "ones1": np.ones((1, 128), dtype=g),
    }


def kernel(**inputs):
    inputs = {k: np.asarray(v, dtype=np.float32) for k, v in inputs.items()}
    if "nc" not in _CACHE:
        _CACHE["nc"] = build_nc()
    nc = _CACHE["nc"]
    in_maps = [_prep_core_inputs(inputs, c) for c in range(N_CORES)]
    trace = bool(int(os.environ.get("KERNEL_TRACE", "0")))
    res = bass_utils.run_bass_kernel_spmd(
        nc,
        in_maps,
        core_ids=list(range(N_CORES)),
        trace=trace,
        tmpdir=os.environ.get("KERNEL_TRACE_DIR") or None,
    )
    _CACHE["last_result"] = res
    out = np.concatenate(
        [res.results[c]["out"].reshape(B, T, D) for c in range(N_CORES)], axis=0
    )
    return out


# revision 26
# speedup vs baseline: 1.0006x; 1.0006x over previous
"""Trainium2 Bass kernel for nn_ItemAutoencoder (LSTM autoencoder).

Model: x[B,T,D] -> relu(x @ in_W.T + in_b)            [B,T,64]
         -> LSTM(64->256) -> LSTM(256->256)            [B,T,256]
         -> z = h[:, -1]                               [B,256]
         -> repeat z over T -> LSTM(256->64) -> LSTM(64->64)
         -> out = d @ out_W.T + out_b                  [B,T,256]
B=1024, T=100, D=256.  Sharding: data-parallel, batch 128 per core x 8 cores.

Per-core structure (all matmuls bf16):
  - Recurrent state h kept TRANSPOSED in bf16 ([128, H] tiles, H rows packed
    into 2x128-col chunks) as the stationary operand of the gate matmuls:
    gates[B, 4H] = hT.T @ W.
  - Gate order host-permuted to [f, i, o, g]: sigmoid covers one contiguous
    col range, tanh the tail.
  - Encoder L0 bias rides a ones row in the in-proj output (K=65).
    Encoder L1 bias is pre-written into PSUM by the DVE (TensorCopy from a
    host-shipped [128, 4H] f32 broadcast tile); the gate matmuls then
    accumulate with start=False.
  - Elementwise ops use scalar_tensor_tensor (2x DVE rate, all-SBUF fp32).
  - h is produced in bf16, transposed on the PE in bf16 (1 cycle/row vs 4 for
    fp32), and copied PSUM->SBUF by one DVE TensorCopy (2x, 2-byte).
  - Decoder: L0 input (z@dW0i + b, const over t) re-injected per step via an
    identity matmul; dec L1/out biases ride ones rows in the dT state tiles.
    Output projection DMAs straight from PSUM to DRAM each step.
"""
import os
import numpy as np
from contextlib import ExitStack

import concourse.bass as bass
import concourse.tile as tile
from concourse import bacc, mybir
from concourse import bass_utils

F32 = mybir.dt.float32
BF16 = mybir.dt.bfloat16
import ml_dtypes
MM_DT = BF16
MM_NP = ml_dtypes.bfloat16
AF = mybir.ActivationFunctionType
ALU = mybir.AluOpType
TS = bass.ts

N_CORES = 8
B = 128            # per-core batch
T = 100
D = 256
H = 256            # encoder hidden
M = 64             # in-proj dim / decoder hidden
GE = 4 * H         # 1024
GD = 4 * M         # 256

_CACHE: dict = {}


def build_nc():
    nc = bacc.Bacc("TRN2", target_bir_lowering=False, debug=False)

    # ---- DRAM I/O -----------------------------------------------------------
    xT_d = nc.dram_tensor("xT", [2, 128, T * B], MM_DT, kind="ExternalInput")
    inWT_d = nc.dram_tensor("inWT", [2, 128, M], MM_DT, kind="ExternalInput")
    inb_d = nc.dram_tensor("inb", [M, 1], F32, kind="ExternalInput")
    w0in_d = nc.dram_tensor("w0in", [M + 1, GE], MM_DT, kind="ExternalInput")
    w0rec_d = nc.dram_tensor("w0rec", [2, 128, GE], MM_DT, kind="ExternalInput")
    w1in_d = nc.dram_tensor("w1in", [2, 128, GE], MM_DT, kind="ExternalInput")
    w1rec_d = nc.dram_tensor("w1rec", [2, 128, GE], MM_DT, kind="ExternalInput")
    b1bc_d = nc.dram_tensor("b1bc", [128, GE], F32, kind="ExternalInput")
    dw0in_d = nc.dram_tensor("dw0in", [2, 128, GD], MM_DT, kind="ExternalInput")
    bd0_d = nc.dram_tensor("bd0", [1, GD], MM_DT, kind="ExternalInput")
    dw0rec_d = nc.dram_tensor("dw0rec", [M, GD], MM_DT, kind="ExternalInput")
    dw1in_d = nc.dram_tensor("dw1in", [M + 1, GD], MM_DT, kind="ExternalInput")
    dw1rec_d = nc.dram_tensor("dw1rec", [M, GD], MM_DT, kind="ExternalInput")
    wout_d = nc.dram_tensor("wout", [M + 1, D], MM_DT, kind="ExternalInput")
    identr_d = nc.dram_tensor("identr", [128, 128], MM_DT, kind="ExternalInput")
    ones1_d = nc.dram_tensor("ones1", [1, 128], MM_DT, kind="ExternalInput")
    out_d = nc.dram_tensor("out", [B, T * D], F32, kind="ExternalOutput")

    with tile.TileContext(nc) as tc, ExitStack() as ctx:
        P = bass.MemorySpace.PSUM
        wp = ctx.enter_context(tc.tile_pool(name="w", bufs=1))

        def wtile(dram_ap, shape, tag, dt=MM_DT):
            t_ = wp.tile(shape, dt, tag=tag)
            nc.sync.dma_start(t_[:], dram_ap)
            return t_

        # ---- persistent weights in SBUF ------------------------------------
        inWT = [wtile(inWT_d[kb, :, :], [128, M], f"inWT{kb}") for kb in range(2)]
        inb = wtile(inb_d[:], [M, 1], "inb", F32)
        w0in = wtile(w0in_d[:], [M + 1, GE], "w0in")
        w0rec = [wtile(w0rec_d[kb, :, :], [128, GE], f"w0rec{kb}") for kb in range(2)]
        w1in = [wtile(w1in_d[kb, :, :], [128, GE], f"w1in{kb}") for kb in range(2)]
        w1rec = [wtile(w1rec_d[kb, :, :], [128, GE], f"w1rec{kb}") for kb in range(2)]
        b1bc = wtile(b1bc_d[:], [128, GE], "b1bc", F32)
        dw0in = [wtile(dw0in_d[kb, :, :], [128, GD], f"dw0in{kb}") for kb in range(2)]
        bd0 = wtile(bd0_d[:], [1, GD], "bd0")
        dw0rec = wtile(dw0rec_d[:], [M, GD], "dw0rec")
        dw1in = wtile(dw1in_d[:], [M + 1, GD], "dw1in")
        dw1rec = wtile(dw1rec_d[:], [M, GD], "dw1rec")
        wout = wtile(wout_d[:], [M + 1, D], "wout")
        identr = wtile(identr_d[:], [128, 128], "identr")
        ones1 = wtile(ones1_d[:], [1, 128], "ones1")

        # in-proj output, transposed, with a ones row (row 64) for bias riding
        h0aug_h = nc.alloc_sbuf_tensor("h0aug", [M + 1, T * B], MM_DT)
        h0aug = h0aug_h.ap()
        nc.gpsimd.memset(h0aug[M : M + 1, :], 1.0)

        # ============================= in-proj ==============================
        xpool = ctx.enter_context(tc.tile_pool(name="xc", bufs=4))
        with tc.tile_pool(name="psip", bufs=2, space=P) as psip:
            NG = T * B // 512  # 25
            for g in range(NG):
                xa = xpool.tile([128, 512], MM_DT, tag="xa")
                xb = xpool.tile([128, 512], MM_DT, tag="xb")
                nc.sync.dma_start(xa[:], xT_d[0, :, TS(g, 512)])
                nc.sync.dma_start(xb[:], xT_d[1, :, TS(g, 512)])
                ps = psip.tile([M, 512], F32)
                nc.tensor.matmul(ps[:], inWT[0][:], xa[:], start=True, stop=False)
                nc.tensor.matmul(ps[:], inWT[1][:], xb[:], start=False, stop=True)
                nc.scalar.activation(
                    h0aug[0:M, TS(g, 512)], ps[:], AF.Relu, bias=inb[:, 0:1]
                )

        # ============================= encoder ==============================
        gpool = ctx.enter_context(tc.tile_pool(name="g", bufs=2))
        spool = ctx.enter_context(tc.tile_pool(name="s", bufs=2))

        # persistent state tiles: [A, B, Z] rotation, Z = zero-init (t=-1)
        def zstate(tag, shape, dt):
            t_ = nc.alloc_sbuf_tensor(tag, shape, dt).ap()
            nc.gpsimd.memset(t_[:], 0.0)
            return t_

        hT = {
            l: [zstate(f"h{l}T{i}", [128, H], MM_DT) for i in range(3)]
            for l in range(2)
        }
        cst = {l: [zstate(f"c{l}_{i}", [128, H], F32) for i in range(3)] for l in range(2)}

        with (
            tc.tile_pool(name="pg0", bufs=1, space=P) as pg0p,
            tc.tile_pool(name="pg1", bufs=2, space=P) as pg1p,
            tc.tile_pool(name="ptr", bufs=2, space=P) as ptrp,
        ):
            pg = {0: pg0p.tile([128, GE], F32, tag="pg0", name="pg0"), 1: None}

            st = {0: {}, 1: {}}

            def enc_prefill_b1():
                # write L1 bias into PSUM; gate matmuls accumulate on top.
                # bufs=2 rotation: this only waits on sigma reads from 2 slots ago,
                # giving the (slow, idle) gpsimd engine a full slot of slack.
                pg[1] = pg1p.tile([128, GE], F32, tag="pg1", name="pg1")
                nc.vector.tensor_copy(pg[1][:], b1bc[:])

            def enc_mms(l, t):
                ps = pg[l]
                if l == 0:
                    srcs = [(h0aug[:, TS(t, 128)], w0in)]
                    hprev = hT[0][(t - 1) % 3 if t >= 1 else 2]
                    wrec = w0rec
                else:
                    hin = hT[0][t % 3]
                    srcs = [(hin[:, 0:128], w1in[0]), (hin[:, 128:256], w1in[1])]
                    hprev = hT[1][(t - 1) % 3 if t >= 1 else 2]
                    wrec = w1rec
                srcs += [(hprev[:, 0:128], wrec[0]), (hprev[:, 128:256], wrec[1])]
                first = l == 0  # L1 accumulates onto DVE-prefilled bias
                for chunk in range(2):
                    ns = slice(chunk * 512, (chunk + 1) * 512)
                    for j, (lh, rh) in enumerate(srcs):
                        nc.tensor.matmul(
                            ps[:, ns],
                            lh,
                            rh[:, ns],
                            start=(j == 0 and first),
                            stop=(j == len(srcs) - 1),
                            skip_group_check=not first,
                        )

            def enc_act_gates(l):
                ps = pg[l]
                gsb = gpool.tile([128, GE], F32, tag=f"gsb{l}")
                st[l]["gsb"] = gsb
                # layout [f, i, o, g]: split sigmoid so f lands early
                nc.scalar.activation(gsb[:, 0 : 2 * H], ps[:, 0 : 2 * H], AF.Sigmoid)
                nc.scalar.activation(gsb[:, 3 * H :], ps[:, 3 * H :], AF.Tanh)
                nc.scalar.activation(gsb[:, 2 * H : 3 * H], ps[:, 2 * H : 3 * H], AF.Sigmoid)

            def enc_dve_c(l, t):
                s = st[l]
                gsb = s["gsb"]
                c_prev = cst[l][(t - 1) % 3 if t >= 1 else 2]
                c_new = cst[l][t % 3]
                ctmp = spool.tile([128, H], F32, tag=f"ctmp{l}")
                u = spool.tile([128, H], F32, tag=f"u{l}")
                nc.vector.scalar_tensor_tensor(
                    ctmp[:], gsb[:, 0:H], 1.0, c_prev[:], ALU.mult, ALU.mult
                )
                nc.vector.scalar_tensor_tensor(
                    u[:], gsb[:, H : 2 * H], 1.0, gsb[:, 3 * H :], ALU.mult, ALU.mult
                )
                nc.vector.scalar_tensor_tensor(
                    c_new[:], ctmp[:], 1.0, u[:], ALU.mult, ALU.add
                )
                s["c"] = c_new

            def enc_act_tanhc(l):
                s = st[l]
                tcs = spool.tile([128, H], F32, tag=f"tc{l}")
                nc.scalar.activation(tcs[:], s["c"][:], AF.Tanh)
                s["tc"] = tcs

            def enc_dve_h(l):
                s = st[l]
                hsb = spool.tile([128, H], MM_DT, tag=f"hsb{l}")
                nc.vector.scalar_tensor_tensor(
                    hsb[:], s["gsb"][:, 2 * H : 3 * H], 1.0, s["tc"][:], ALU.mult, ALU.mult
                )
                s["h"] = hsb

            def enc_pe_tr(l):
                s = st[l]
                ptr = ptrp.tile([128, H], MM_DT, tag="ptr")
                nc.tensor.transpose(ptr[:, 0:128], s["h"][:, 0:128], identr[:])
                nc.tensor.transpose(ptr[:, 128:256], s["h"][:, 128:256], identr[:])
                s["ptr"] = ptr

            def enc_copy_h(l, t):
                s = st[l]
                nc.vector.tensor_copy(hT[l][t % 3][:], s["ptr"][:])

            def enc_slot(work):  # work = list of (layer, t); L0 first
                if any(l == 1 for l, _ in work):
                    enc_prefill_b1()
                for l, t in work:
                    enc_mms(l, t)
                for l, _ in work:
                    enc_act_gates(l)
                for l, t in work:
                    enc_dve_c(l, t)
                for l, _ in work:
                    enc_act_tanhc(l)
                for l, _ in work:
                    enc_dve_h(l)
                for l, _ in work:
                    enc_pe_tr(l)
                for l, t in work:
                    enc_copy_h(l, t)

            for t in range(T):
                work = [(0, t)]
                if t >= 1:
                    work.append((1, t - 1))
                enc_slot(work)
            enc_slot([(1, T - 1)])

        zT = hT[1][(T - 1) % 3]  # [128, 256] bf16

        # ============================= decoder ==============================
        dst = {}
        with (
            tc.tile_pool(name="pd0", bufs=2, space=P) as pd0p,
            tc.tile_pool(name="pd1", bufs=1, space=P) as pd1p,
            tc.tile_pool(name="ptrd", bufs=2, space=P) as ptrdp,
            tc.tile_pool(name="pout", bufs=2, space=P) as poutp,
            tc.tile_pool(name="pxg", bufs=1, space=P) as pxgp,
        ):
            # xg0 = z @ dW0i.T + bd0  (constant over time); kept f32 in SBUF and
            # re-injected into PSUM each step by the gpsimd engine
            psx = pxgp.tile([128, GD], F32)
            nc.tensor.matmul(psx[:], ones1[:], bd0[:], start=True, stop=False)
            nc.tensor.matmul(psx[:], zT[:, 0:128], dw0in[0][:], start=False, stop=False)
            nc.tensor.matmul(psx[:], zT[:, 128:256], dw0in[1][:], start=False, stop=True)
            xg0 = wp.tile([128, GD], F32, tag="xg0")
            nc.scalar.activation(xg0[:], psx[:], AF.Copy)

            # persistent ones-row-augmented transposed decoder states [A,B,Z]
            def dstate(tag):
                t_ = nc.alloc_sbuf_tensor(tag, [M + 1, 128], MM_DT).ap()
                nc.gpsimd.memset(t_[:], 0.0)
                nc.gpsimd.memset(t_[M : M + 1, :], 1.0)
                return t_

            dT = {l: [dstate(f"d{l}T{i}") for i in range(3)] for l in range(2)}
            dcs = {l: [zstate(f"cd{l}_{i}", [128, M], F32) for i in range(3)] for l in range(2)}

            pd = {0: None, 1: pd1p.tile([128, GD], F32, tag="pd1", name="pd1")}

            def dec_prefill_xg0():
                pd[0] = pd0p.tile([128, GD], F32, tag="pd0", name="pd0")
                nc.vector.tensor_copy(pd[0][:], xg0[:])

            def dec_mm_rec0(t):
                dprev = dT[0][(t - 1) % 3 if t >= 1 else 2]
                nc.tensor.matmul(
                    pd[0][:], dprev[0:M, :], dw0rec[:],
                    start=False, stop=True, skip_group_check=True,
                )

            def dec_mm_d1rec(t):
                # d1T(t-1) is ready a full slot early -> emit first, start the group
                dprev = dT[1][(t - 1) % 3 if t >= 1 else 2]
                nc.tensor.matmul(pd[1][:], dprev[0:M, :], dw1rec[:], start=True, stop=False)

            def dec_mm_d1in(t):
                din = dT[0][t % 3]
                nc.tensor.matmul(pd[1][:], din[0 : M + 1, :], dw1in[:], start=False, stop=True)

            def dec_act_gates(l):
                ps = pd[l]
                gsb = gpool.tile([128, GD], F32, tag=f"dgsb{l}")
                dst[l]["gsb"] = gsb
                nc.scalar.activation(gsb[:, 0 : 3 * M], ps[:, 0 : 3 * M], AF.Sigmoid)
                nc.scalar.activation(gsb[:, 3 * M :], ps[:, 3 * M :], AF.Tanh)

            def dec_dve_c(l, t):
                eng = nc.vector
                s = dst[l]
                gsb = s["gsb"]
                c_prev = dcs[l][(t - 1) % 3 if t >= 1 else 2]
                c_new = dcs[l][t % 3]
                ctmp = spool.tile([128, M], F32, tag=f"dctmp{l}")
                u = spool.tile([128, M], F32, tag=f"du{l}")
                eng.scalar_tensor_tensor(
                    ctmp[:], gsb[:, 0:M], 1.0, c_prev[:], ALU.mult, ALU.mult
                )
                eng.scalar_tensor_tensor(
                    u[:], gsb[:, M : 2 * M], 1.0, gsb[:, 3 * M :], ALU.mult, ALU.mult
                )
                eng.scalar_tensor_tensor(
                    c_new[:], ctmp[:], 1.0, u[:], ALU.mult, ALU.add
                )
                s["c"] = c_new

            def dec_act_tanhc(l):
                s = dst[l]
                tcs = spool.tile([128, M], F32, tag=f"dtc{l}")
                nc.scalar.activation(tcs[:], s["c"][:], AF.Tanh)
                s["tc"] = tcs

            def dec_dve_h(l):
                eng = nc.vector
                s = dst[l]
                hsb = spool.tile([128, M], MM_DT, tag=f"dhsb{l}")
                eng.scalar_tensor_tensor(
                    hsb[:], s["gsb"][:, 2 * M : 3 * M], 1.0, s["tc"][:], ALU.mult, ALU.mult
                )
                s["h"] = hsb

            def dec_pe_tr(l):
                s = dst[l]
                ptr = ptrdp.tile([M, 128], MM_DT, tag="ptrd")
                nc.tensor.transpose(ptr[:], s["h"][:], identr[:])
                s["ptr"] = ptr

            def dec_copy_h(l, t):
                s = dst[l]
                nc.vector.tensor_copy(dT[l][t % 3][0:M, :], s["ptr"][:])

            ochunk = {"tile": None}
            oproj = {}

            def outproj_mm(t):
                d1T = dT[1][t % 3]
                ps = poutp.tile([128, D], F32, tag="pout")
                nc.tensor.matmul(ps[:], d1T[0 : M + 1, :], wout[:], start=True, stop=True)
                oproj["ps"] = ps

            def outproj_copy(t):
                if t % 10 == 0:
                    ochunk["tile"] = spool.tile([128, 10 * D], F32, tag="ochunk", name="ochunk")
                dest = ochunk["tile"][:, TS(t % 10, D)]
                if t % 2 == 0:
                    nc.scalar.activation(dest, oproj["ps"][:], AF.Copy)
                else:
                    nc.vector.tensor_copy(dest, oproj["ps"][:])
                if t % 10 == 9:
                    nc.sync.dma_start(out_d[:, TS(t // 10, 10 * D)], ochunk["tile"][:])

            def dec_slot(work, op_t, prefill_next=True):
                has0 = any(l == 0 for l, _ in work)
                has1 = any(l == 1 for l, _ in work)
                t0 = next((t for l, t in work if l == 0), None)
                t1 = next((t for l, t in work if l == 1), None)
                for l, _ in work:
                    dst.setdefault(l, {})
                # PE: early-ready operands first, recurrence-critical next
                if has1:
                    dec_mm_d1rec(t1)
                if op_t is not None:
                    outproj_mm(op_t)
                if has0:
                    dec_mm_rec0(t0)
                if has1:
                    dec_mm_d1in(t1)
                for l, _ in work:
                    dec_act_gates(l)
                for l, t in work:
                    dec_dve_c(l, t)
                for l, _ in work:
                    dec_act_tanhc(l)
                for l, _ in work:
                    dec_dve_h(l)
                for l, _ in work:
                    dec_pe_tr(l)
                for l, t in work:
                    dec_copy_h(l, t)
                # housekeeping last: never ahead of chain ops in the queues
                if op_t is not None:
                    outproj_copy(op_t)
                if prefill_next and has0:
                    dec_prefill_xg0()

            dec_prefill_xg0()  # pd0 for slot 0
            for t in range(T):
                work = [(0, t)]
                if t >= 1:
                    work.append((1, t - 1))
                op_t = t - 2 if t >= 2 else None
                dec_slot(work, op_t, prefill_next=(t < T - 1))
            dec_slot([(1, T - 1)], T - 2, prefill_next=False)
            dec_slot([], T - 1, prefill_next=False)

    nc.compile()
    return nc


# ----------------------------------------------------------------------------
# host-side wrapper
# ----------------------------------------------------------------------------

def _perm(n):
    """pytorch gate order i,f,g,o (blocks of n) -> [f, i, o, g]."""
    idx = np.arange(4 * n).reshape(4, n)
    return np.concatenate([idx[1], idx[0], idx[3], idx[2]])


def _prep_core_inputs(inputs, core):
    f = np.float32
    pe = _perm(H)
    pd = _perm(M)
    x = inputs["x"][core * B : (core + 1) * B]          # [128, 100, 256]
    xT = np.ascontiguousarray(x.transpose(2, 1, 0)).reshape(2, 128, T * B)

    w0in = np.concatenate(
        [inputs["eW0i"].T[:, pe], (inputs["eb0i"] + inputs["eb0h"])[None, pe]], 0
    )
    w0rec = inputs["eW0h"].T[:, pe].reshape(2, 128, GE)
    w1in = inputs["eW1i"].T[:, pe].reshape(2, 128, GE)
    w1rec = inputs["eW1h"].T[:, pe].reshape(2, 128, GE)
    b1bc = np.broadcast_to((inputs["eb1i"] + inputs["eb1h"])[None, pe], (128, GE))
    dw0in = inputs["dW0i"].T[:, pd].reshape(2, 128, GD)
    bd0 = (inputs["db0i"] + inputs["db0h"])[None, pd]
    dw0rec = inputs["dW0h"].T[:, pd]
    dw1in = np.concatenate(
        [inputs["dW1i"].T[:, pd], (inputs["db1i"] + inputs["db1h"])[None, pd]], 0
    )
    dw1rec = inputs["dW1h"].T[:, pd]
    wout = np.concatenate([inputs["out_W"].T, inputs["out_b"][None, :]], 0)

    g = MM_NP
    return {
        "xT": np.ascontiguousarray(xT, dtype=g),
        "inWT": np.ascontiguousarray(inputs["in_W"].T.reshape(2, 128, M), dtype=g),
        "inb": np.ascontiguousarray(inputs["in_b"][:, None], dtype=f),
        "w0in": np.ascontiguousarray(w0in, dtype=g),
        "w0rec": np.ascontiguousarray(w0rec, dtype=g),
        "w1in": np.ascontiguousarray(w1in, dtype=g),
        "w1rec": np.ascontiguousarray(w1rec, dtype=g),
        "b1bc": np.ascontiguousarray(b1bc, dtype=f),
        "dw0in": np.ascontiguousarray(dw0in, dtype=g),
        "bd0": np.ascontiguousarray(bd0, dtype=g),
        "dw0rec": np.ascontiguousarray(dw0rec, dtype=g),
        "dw1in": np.ascontiguousarray(dw1in, dtype=g),
        "dw1rec": np.ascontiguousarray(dw1rec, dtype=g),
        "wout": np.ascontiguousarray(wout, dtype=g),
        "identr": np.eye(128).astype(g),
        "ones1": np.ones((1, 128), dtype=g),
    }


def kernel(**inputs):
    inputs = {k: np.asarray(v, dtype=np.float32) for k, v in inputs.items()}
    if "nc" not in _CACHE:
        _CACHE["nc"] = build_nc()
    nc = _CACHE["nc"]
    in_maps = [_prep_core_inputs(inputs, c) for c in range(N_CORES)]
    trace = bool(int(os.environ.get("KERNEL_TRACE", "0")))
    res = bass_utils.run_bass_kernel_spmd(
        nc,
        in_maps,
        core_ids=list(range(N_CORES)),
        trace=trace,
        tmpdir=os.environ.get("KERNEL_TRACE_DIR") or None,
    )
    _CACHE["last_result"] = res
    out = np.concatenate(
        [res.results[c]["out"].reshape(B, T, D) for c in range(N_CORES)], axis=0
    )
    return out
